# revision 20
# baseline (speedup 1.0000x reference)
"""Two-layer GCN (PyG GCNConv x2 + rrelu) on 8 Trainium2 NeuronCores.

Math: with A = adjacency-with-multiplicity + I (self loops), deg = in-degree
(including the self loop), dinv = deg^-1/2:
    z1[v] = dinv[v] * (sum_{u->v} dinv[u]*x[u]) @ W1 + b1
    g[u]  = dinv[u] * rrelu(z1[u])                      (dinv pre-folded for L2)
    z2[v] = dinv[v] * (sum_{u->v} g[u]) @ W2 + b2
Aggregation is linear, so the dense W matmul is applied post-aggregation on
the [128, 128] per-destination-block aggregate -- 128x less PE work than
transforming every edge message.

Sharding: destinations are range-sharded across the 8 cores (12544 each).
Every core keeps a replicated (dinv-prescaled, bf16) source-feature table in
its own HBM and fetches the source rows of its edges with dma_gather (int16
indices -> four 25088-row source windows).  The SWDGE descriptor-generation
fixed cost (~1us/call) dominated the previous version (one call per
(block, window) = 392/layer), so gathers are batched to ONE call per
(superblock of 7 blocks, window) = 56/layer with a window-major msg layout
so each call writes contiguous columns.

Per destination block of 128 nodes, gathered edge-message chunks
[128 edges, 128 feat] are scatter-reduced on the TensorEngine by matmul with
one-hot selectors Sel[e, dest] = (d[e] == dest) generated on-device
(is_equal with broadcast operand).  Self-loop contributions bypass the
gather: their source rows are contiguous, so a plain DMA + identity matmul
adds them.  The dense W matmul uses the f32->bf16-cast aggregate as the
STATIONARY operand so z comes out dest-major [dest, outF]; dinv[dest] is
then a per-partition scalar, letting the otherwise idle Scalar engine apply
the dinv scaling and rrelu (Prelu activation) off the Vector engine's
critical path.  Outputs are written dest-major = node-major, so the host
never transposes.  Two NEFF dispatches (layer 1, layer 2).

The harness calls kernel(**inputs) with full inputs; index bucketing,
program build, compile, SPMD run on cores 0-7 and unshard all happen here.
"""

import sys

for _p in ("/opt/trn_rl_repo",):
    if _p not in sys.path:
        sys.path.insert(0, _p)

import numpy as np
import ml_dtypes

import concourse.bacc as bacc
import concourse.bass as bass
import concourse.mybir as mybir
import concourse.tile as tile
from concourse.bass_utils import run_bass_kernel_spmd

P = 128  # partition width == dest block width == feature width
RRELU_SLOPE = (1.0 / 8.0 + 1.0 / 3.0) / 2.0
MAX_CALL_COLS = 7   # dma_gather is capped at 1008 indices per call


def _call_plan(win_cols):
    """Per-superblock gather calls [(window, col0, ncols)], chunked to
    MAX_CALL_COLS and round-robin interleaved across windows so the four
    SWDGE queues fill evenly (a queue's ring holds only ~2 calls; emitting
    one window's calls back-to-back blocks GpSimd and starves the rest)."""
    pending = [(k, 0, win_cols[k]) for k in range(len(win_cols))]
    plan = []
    while pending:
        nxt = []
        for (k, c0, total) in pending:
            ncols = min(MAX_CALL_COLS, total - c0)
            plan.append((k, c0, ncols))
            if c0 + ncols < total:
                nxt.append((k, c0 + ncols, total))
        pending = nxt
    return plan


class Cfg:
    def __init__(self, n_nodes, n_cores, blocks_per_core, superblock, in_f,
                 out1_f, out2_f, src_window, dense_cols=4):
        self.n_nodes = n_nodes
        self.n_cores = n_cores
        self.bpc = blocks_per_core            # dest blocks per core
        self.sb = superblock                  # blocks per superblock
        assert blocks_per_core % superblock == 0
        self.sb_count = blocks_per_core // superblock
        self.in_f = in_f
        self.out1_f = out1_f
        self.out2_f = out2_f
        self.src_window = src_window          # int16 gather range per window
        self.dense_cols = dense_cols          # private cols per (block, win)
        self.nodes_per_core = blocks_per_core * P
        self.n_pad = n_cores * self.nodes_per_core
        assert self.n_pad >= n_nodes
        assert src_window % P == 0 and src_window <= 32768
        self.n_chunks = -(-self.n_pad // src_window)
        self.tab_rows = self.n_chunks * src_window


FULL = Cfg(n_nodes=100000, n_cores=8, blocks_per_core=98, superblock=7,
           in_f=128, out1_f=128, out2_f=64, src_window=25088, dense_cols=4)


# --------------------------------------------------------------------------
# host-side index preprocessing
# --------------------------------------------------------------------------

def preprocess(edge_index, cfg):
    """Bucket edges by (dest block, src window); self loops are handled
    separately on-device.  Build per-core gather index / dest-local tables
    and the degree scaling.

    Column layout (per superblock, window-major):
      [win k: sb*DENSE dense cols | ovf[k] overflow cols]  for k in windows
    Each (block, window) gets DENSE=4 private columns (512 slots); edges
    beyond 512 spill into the window's shared overflow columns, which are
    scatter-matmul'ed once per block with a per-block d_tab column (non-own
    edges masked to -1).  This cuts gather padding from ~20% to a few %
    while keeping PE/DVE work per block unchanged."""
    row = edge_index[0].astype(np.int64)
    col = edge_index[1].astype(np.int64)
    n = cfg.n_nodes

    deg = np.bincount(col, minlength=cfg.n_pad).astype(np.float64) + 1.0
    dinv = (1.0 / np.sqrt(deg)).astype(np.float32)
    dinv[n:] = 1.0

    blk = col >> 7                      # global dest block
    chunk = row // cfg.src_window
    order = np.lexsort((chunk, blk))
    row, col, blk, chunk = row[order], col[order], blk[order], chunk[order]

    n_blocks = cfg.n_cores * cfg.bpc
    counts = np.zeros((n_blocks, cfg.n_chunks), dtype=np.int64)
    np.add.at(counts, (blk, chunk), 1)

    bc_start = np.zeros(n_blocks * cfg.n_chunks + 1, dtype=np.int64)
    np.cumsum(counts.reshape(-1), out=bc_start[1:])

    DENSE = cfg.dense_cols
    dn = DENSE * P
    # overflow slots per (core, superblock, window) -> uniform col caps
    ovf_cnt = np.zeros((cfg.n_cores, cfg.sb_count, cfg.n_chunks), dtype=np.int64)
    for c in range(cfg.n_cores):
        for s in range(cfg.sb_count):
            for k in range(cfg.n_chunks):
                tot = 0
                for b7 in range(cfg.sb):
                    b_glob = c * cfg.bpc + s * cfg.sb + b7
                    tot += max(0, counts[b_glob, k] - dn)
                ovf_cnt[c, s, k] = tot
    ovf_cols = [int(-(-int(ovf_cnt[:, :, k].max()) // P)) for k in range(cfg.n_chunks)]
    c_blk = cfg.n_chunks * DENSE + sum(ovf_cols)   # d_tab cols per block
    win_cols = [cfg.sb * DENSE + ovf_cols[k] for k in range(cfg.n_chunks)]
    wbase = np.concatenate([[0], np.cumsum(win_cols)]).astype(int)
    sb_cols = int(wbase[-1])                       # msg cols per superblock

    per_core = []
    for c in range(cfg.n_cores):
        d_tab = np.full((P, cfg.bpc * c_blk), -1.0, dtype=np.float64)
        # per (s, k): dense idx [sb*dn], ovf idx [ovf_cols[k]*P]
        dense_idx = np.zeros((cfg.sb_count, cfg.n_chunks, cfg.sb * dn), np.int64)
        ovf_idx = [np.zeros((cfg.sb_count, ovf_cols[k] * P), np.int64)
                   for k in range(cfg.n_chunks)]
        for s in range(cfg.sb_count):
            for k in range(cfg.n_chunks):
                ov_rows, ov_dest, ov_blk7 = [], [], []
                for b7 in range(cfg.sb):
                    b_loc = s * cfg.sb + b7
                    b_glob = c * cfg.bpc + b_loc
                    lo = bc_start[b_glob * cfg.n_chunks + k]
                    hi = bc_start[b_glob * cfg.n_chunks + k + 1]
                    r_all = row[lo:hi] - k * cfg.src_window
                    d_all = col[lo:hi] - b_glob * P
                    nd = min(len(r_all), dn)
                    seg = np.zeros(dn, dtype=np.int64)
                    seg[:nd] = r_all[:nd]
                    if nd < dn:
                        seg[nd:] = seg[0] if nd > 0 else 0
                    dense_idx[s, k, b7 * dn:(b7 + 1) * dn] = seg
                    d_seg = np.full(dn, -1.0)
                    d_seg[:nd] = d_all[:nd].astype(np.float64)
                    gcol0 = b_loc * c_blk + k * DENSE
                    d_tab[:, gcol0:gcol0 + DENSE] = d_seg.reshape(DENSE, P).T
                    if len(r_all) > dn:
                        ov_rows.append(r_all[dn:])
                        ov_dest.append(d_all[dn:])
                        ov_blk7.append(np.full(len(r_all) - dn, b7))
                vcols = ovf_cols[k]
                if vcols == 0:
                    continue
                vslots = vcols * P
                if ov_rows:
                    orow = np.concatenate(ov_rows)
                    odst = np.concatenate(ov_dest)
                    ob7 = np.concatenate(ov_blk7)
                else:
                    orow = np.zeros(0, np.int64)
                    odst = np.zeros(0, np.int64)
                    ob7 = np.zeros(0, np.int64)
                cnt = len(orow)
                assert cnt <= vslots, (cnt, vslots)
                oseg = np.zeros(vslots, dtype=np.int64)
                oseg[:cnt] = orow
                if cnt < vslots:
                    oseg[cnt:] = oseg[0] if cnt > 0 else 0
                ovf_idx[k][s] = oseg
                # per-block d columns for the shared overflow cols
                for b7 in range(cfg.sb):
                    b_loc = s * cfg.sb + b7
                    dv = np.full(vslots, -1.0)
                    mine = ob7 == b7
                    dv[:cnt][mine] = odst[mine].astype(np.float64)
                    gcol0 = b_loc * c_blk + cfg.n_chunks * DENSE + int(np.sum(ovf_cols[:k]))
                    d_tab[:, gcol0:gcol0 + vcols] = dv.reshape(vcols, P).T
        # idx table in call-emission order (round-robin interleaved over
        # windows so all SWDGE queues stay fed -- see _call_plan)
        plan = _call_plan(win_cols)
        idx_parts = []
        for s in range(cfg.sb_count):
            win_flat = [np.concatenate([dense_idx[s, k],
                                        ovf_idx[k][s]])
                        for k in range(cfg.n_chunks)]
            for (k, c0, ncols) in plan:
                idx_parts.append(
                    win_flat[k][c0 * P:(c0 + ncols) * P].astype(np.int16))
        idx_flat = [a.reshape(-1, 16).T for a in idx_parts]
        idx_tab = np.concatenate(idx_flat, axis=1)
        idx_tab = np.tile(idx_tab, (8, 1))          # [128, total/16]
        # dinv columns: dinv_cols[p, b] = dinv[core_base + b*128 + p]
        dslice = dinv[c * cfg.nodes_per_core:(c + 1) * cfg.nodes_per_core]
        dinv_cols = np.ascontiguousarray(dslice.reshape(cfg.bpc, P).T)
        per_core.append({
            "idx_tab": np.ascontiguousarray(idx_tab),
            "d_tab": np.ascontiguousarray(d_tab.astype(ml_dtypes.bfloat16)),
            "dinv_cols": dinv_cols,
        })

    return {"ovf_cols": ovf_cols, "c_blk": c_blk, "dinv": dinv,
            "per_core": per_core}


# --------------------------------------------------------------------------
# bass program (one GCN layer, SPMD across cores; all data via inputs)
# --------------------------------------------------------------------------

def build_layer_program(cfg, ovf_cols, layer, has_bias=False):
    """layer=1: out = bf16 g [nodes_per_core, 128]  (dinv*rrelu(z1), node-major)
       layer=2: out = f32  z2 [nodes_per_core, out2_f]"""
    DENSE = cfg.dense_cols
    ovf_cols = [int(x) for x in ovf_cols]
    c_blk = cfg.n_chunks * DENSE + sum(ovf_cols)
    win_cols = [cfg.sb * DENSE + ovf_cols[k] for k in range(cfg.n_chunks)]
    wbase = [0]
    for wc in win_cols:
        wbase.append(wbase[-1] + wc)
    sb_cols = wbase[-1]                      # msg cols per superblock
    plan = _call_plan(win_cols)
    # per block: (msg col, d_tab col-within-block) in enumeration order
    def block_cols(b7):
        cols = []
        for k in range(cfg.n_chunks):
            for cd in range(DENSE):
                cols.append((wbase[k] + b7 * DENSE + cd, k * DENSE + cd))
        for k in range(cfg.n_chunks):
            for v in range(ovf_cols[k]):
                cols.append((wbase[k] + cfg.sb * DENSE + v,
                             cfg.n_chunks * DENSE + sum(ovf_cols[:k]) + v))
        return cols
    out_f = cfg.out1_f if layer == 1 else cfg.out2_f
    out_dt = mybir.dt.bfloat16 if layer == 1 else mybir.dt.float32
    idx_cols_sb = sb_cols * P // 16          # idx free-dim per superblock
    G = 8                                     # sel-gen chunk group width

    nc = bacc.Bacc("TRN2", target_bir_lowering=False, debug=False,
                   num_devices=cfg.n_cores,
                   num_swdge_queues=min(4, cfg.n_chunks))
    dt = mybir.dt
    src_tab = nc.dram_tensor("src_tab", [cfg.tab_rows, P], dt.bfloat16,
                             kind="ExternalInput")
    w_in = nc.dram_tensor("w", [P, out_f], dt.bfloat16, kind="ExternalInput")
    dinv_in = nc.dram_tensor("dinv_cols", [P, cfg.bpc], dt.float32,
                             kind="ExternalInput")
    idx_in = nc.dram_tensor("idx_tab", [P, cfg.sb_count * idx_cols_sb], dt.int16,
                            kind="ExternalInput")
    d_in = nc.dram_tensor("d_tab", [P, cfg.bpc * c_blk], dt.bfloat16,
                          kind="ExternalInput")
    iota_in = nc.dram_tensor("iota", [P, G * P], dt.bfloat16, kind="ExternalInput")
    ident_in = nc.dram_tensor("ident", [P, P], dt.bfloat16, kind="ExternalInput")
    out_t = nc.dram_tensor("out_t", [cfg.nodes_per_core, out_f], out_dt,
                           kind="ExternalOutput")
    # per-core self-loop source rows, staged by the host (node-major slice of
    # src_tab rows owned by this core; avoids needing the core id on device)
    self_in = nc.dram_tensor("self_rows", [cfg.nodes_per_core, P], dt.bfloat16,
                             kind="ExternalInput")
    if has_bias:
        bias_in = nc.dram_tensor("bias_full", [P, out_f], dt.float32,
                                 kind="ExternalInput")

    with tile.TileContext(nc) as tc:
        with (
            tc.tile_pool(name="const", bufs=1) as const_pool,
            tc.tile_pool(name="idx", bufs=3) as idx_pool,
            tc.tile_pool(name="msg", bufs=3) as msg_pool,
            tc.tile_pool(name="selfp", bufs=3) as self_pool,
            tc.tile_pool(name="sel", bufs=6) as sel_pool,
            tc.tile_pool(name="aggsb", bufs=3) as aggsb_pool,
            tc.tile_pool(name="tmp", bufs=3) as tmp_pool,
            tc.tile_pool(name="outsb", bufs=2) as out_pool,
            tc.tile_pool(name="psA", bufs=3, space="PSUM") as agg_psum,
            tc.tile_pool(name="psZ", bufs=2, space="PSUM") as z_psum,
        ):
            w_sb = const_pool.tile([P, out_f], dt.bfloat16)
            nc.sync.dma_start(out=w_sb[:], in_=w_in[:])
            dinv_sb = const_pool.tile([P, cfg.bpc], dt.float32)
            nc.sync.dma_start(out=dinv_sb[:], in_=dinv_in[:])
            iota_sb = const_pool.tile([P, G * P], dt.bfloat16)
            nc.sync.dma_start(out=iota_sb[:], in_=iota_in[:])
            ident_sb = const_pool.tile([P, P], dt.bfloat16)
            nc.sync.dma_start(out=ident_sb[:], in_=ident_in[:])
            d_sb = const_pool.tile([P, cfg.bpc * c_blk], dt.bfloat16)
            nc.sync.dma_start(out=d_sb[:], in_=d_in[:])
            alpha_sb = const_pool.tile([P, 1], dt.float32)
            nc.vector.memset(alpha_sb[:], float(RRELU_SLOPE))
            if has_bias:
                bias_sb = const_pool.tile([P, out_f], dt.float32)
                nc.sync.dma_start(out=bias_sb[:], in_=bias_in[:])

            self_view = self_in.rearrange("(s b p) f -> s p b f",
                                          p=P, b=cfg.sb)
            out_view = out_t.rearrange("(s b p) f -> s p b f",
                                       p=P, b=cfg.sb)

            def finish_block(b_loc, aggsb, out_sb, b7):
                """W matmul (agg stationary -> z dest-major) + ACT epilogue."""
                zps = z_psum.tile([P, out_f], dt.float32)
                nc.tensor.matmul(zps[:], lhsT=aggsb[:], rhs=w_sb[:],
                                 start=True, stop=True)
                dcol = dinv_sb[:, b_loc:b_loc + 1]
                o_sl = out_sb[:, b7, :]
                if layer == 1:
                    t1 = tmp_pool.tile([P, out_f], dt.float32, tag="t1")
                    if has_bias:
                        tz = tmp_pool.tile([P, out_f], dt.float32, tag="tz")
                        nc.scalar.activation(
                            tz[:], zps[:], mybir.ActivationFunctionType.Copy,
                            scale=dcol)
                        tb = tmp_pool.tile([P, out_f], dt.float32, tag="tb")
                        nc.vector.tensor_tensor(tb[:], tz[:], bias_sb[:],
                                                mybir.AluOpType.add)
                        nc.scalar.activation(
                            t1[:], tb[:], mybir.ActivationFunctionType.Prelu,
                            scale=1.0, alpha=alpha_sb[:, 0:1])
                    else:
                        nc.scalar.activation(
                            t1[:], zps[:], mybir.ActivationFunctionType.Prelu,
                            scale=dcol, alpha=alpha_sb[:, 0:1])
                    nc.scalar.activation(
                        o_sl, t1[:], mybir.ActivationFunctionType.Copy,
                        scale=dcol)
                else:
                    if has_bias:
                        tz = tmp_pool.tile([P, out_f], dt.float32, tag="tz")
                        nc.scalar.activation(
                            tz[:], zps[:], mybir.ActivationFunctionType.Copy,
                            scale=dcol)
                        nc.vector.tensor_tensor(o_sl, tz[:], bias_sb[:],
                                                mybir.AluOpType.add)
                    else:
                        nc.scalar.activation(
                            o_sl, zps[:], mybir.ActivationFunctionType.Copy,
                            scale=dcol)

            for s in range(cfg.sb_count):
                idx_sb = idx_pool.tile([P, idx_cols_sb], dt.int16)
                nc.sync.dma_start(
                    out=idx_sb[:],
                    in_=idx_in[:, s * idx_cols_sb:(s + 1) * idx_cols_sb])
                # contiguous self-loop rows for this superblock
                selfs = self_pool.tile([P, cfg.sb, P], dt.bfloat16)
                nc.sync.dma_start(out=selfs[:], in_=self_view[s])

                # gather calls per window (window-major msg cols), chunked to
                # MAX_CALL_COLS columns and interleaved across queues
                msg = msg_pool.tile([P, sb_cols, P], dt.bfloat16)
                off = 0
                for (k, c0, ncols) in plan:
                    mcol0 = wbase[k] + c0
                    n_idx = ncols * P
                    nc.gpsimd.dma_gather(
                        msg[:, mcol0:mcol0 + ncols, :],
                        src_tab[k * cfg.src_window:
                                (k + 1) * cfg.src_window, :],
                        idx_sb[:, off:off + n_idx // 16],
                        n_idx, n_idx, P,
                        queue_num=k % 4,
                    )
                    off += n_idx // 16

                out_sb = out_pool.tile([P, cfg.sb, out_f], out_dt)
                pending = None  # (b_loc, aggsb, b7) 1-deep pipeline
                for b7 in range(cfg.sb):
                    b_loc = s * cfg.sb + b7
                    dcol0 = b_loc * c_blk
                    cols = block_cols(b7)
                    sels = []
                    done = 0
                    while done < c_blk:
                        g = min(G, c_blk - done)
                        sel = sel_pool.tile([P, G * P], dt.bfloat16)
                        nc.vector.tensor_tensor(
                            sel[:, :g * P],
                            iota_sb[:, :g * P],
                            d_sb[:, dcol0 + done:dcol0 + done + g]
                                .to_broadcast([P, g, P]),
                            mybir.AluOpType.is_equal,
                        )
                        sels.extend((sel, j) for j in range(g))
                        done += g

                    agg = agg_psum.tile([P, P], dt.float32)
                    for ci, (sel, j) in enumerate(sels):
                        mcol = cols[ci][0]
                        nc.tensor.matmul(
                            agg[:],
                            lhsT=msg[:, mcol, :],
                            rhs=sel[:, j * P:(j + 1) * P],
                            start=(ci == 0), stop=False,
                        )
                    # self-loop contribution: aggT += selfs[:, b7, :]^T
                    nc.tensor.matmul(
                        agg[:], lhsT=selfs[:, b7, :], rhs=ident_sb[:],
                        start=False, stop=True)

                    aggsb = aggsb_pool.tile([P, P], dt.bfloat16)
                    nc.vector.tensor_copy(aggsb[:], agg[:])

                    if pending is not None:
                        finish_block(*pending)
                    pending = (b_loc, aggsb, out_sb, b7)
                finish_block(*pending)

                nc.sync.dma_start(out=out_view[s], in_=out_sb[:])

    nc.compile()
    return nc


# --------------------------------------------------------------------------
# orchestration
# --------------------------------------------------------------------------

def _iota_tile(G=8):
    return np.tile(np.arange(P, dtype=np.float32), G)[None, :].repeat(P, 0).astype(ml_dtypes.bfloat16)


def _run_gcn(x, edge_index, W1, b1, W2, b2, cfg, runner=None, want_times=False):
    """Shared driver; runner(nc, in_maps) -> list of per-core output dicts."""
    meta = preprocess(np.asarray(edge_index), cfg)
    dinv = meta["dinv"]
    npc = cfg.nodes_per_core

    if runner is None:
        times = []

        def runner(nc, in_maps):
            r = run_bass_kernel_spmd(nc, in_maps, core_ids=list(range(cfg.n_cores)),
                                     trace=want_times)
            if want_times:
                times.append(r.exec_time_ns)
            return r.results
    else:
        times = None

    x = np.asarray(x, dtype=np.float32)
    xs = np.zeros((cfg.tab_rows, P), dtype=ml_dtypes.bfloat16)
    xs[:cfg.n_nodes] = (x * dinv[:cfg.n_nodes, None]).astype(ml_dtypes.bfloat16)

    iota = _iota_tile()
    ident = np.eye(P, dtype=np.float32).astype(ml_dtypes.bfloat16)
    w1 = np.asarray(W1, np.float32).astype(ml_dtypes.bfloat16)
    w2 = np.asarray(W2, np.float32).astype(ml_dtypes.bfloat16)
    b1c = np.asarray(b1, np.float32).reshape(-1)
    b2c = np.asarray(b2, np.float32).reshape(-1)
    hb1 = bool(np.any(b1c != 0.0))
    hb2 = bool(np.any(b2c != 0.0))

    nc1 = build_layer_program(cfg, meta["ovf_cols"], layer=1, has_bias=hb1)
    in_maps = [
        {"src_tab": xs, "w": w1, "iota": iota, "ident": ident,
         "self_rows": np.ascontiguousarray(xs[c * npc:(c + 1) * npc]),
         **{k: pc[k] for k in ("idx_tab", "d_tab", "dinv_cols")}}
        for c, pc in enumerate(meta["per_core"])
    ]
    if hb1:
        bf = np.ascontiguousarray(np.broadcast_to(b1c, (P, cfg.out1_f)).astype(np.float32))
        for m in in_maps:
            m["bias_full"] = bf
    res1 = runner(nc1, in_maps)

    gs = np.zeros((cfg.tab_rows, P), dtype=ml_dtypes.bfloat16)
    for c in range(cfg.n_cores):
        gs[c * npc:(c + 1) * npc] = res1[c]["out_t"]

    nc2 = build_layer_program(cfg, meta["ovf_cols"], layer=2, has_bias=hb2)
    for c in range(cfg.n_cores):
        in_maps[c] = dict(in_maps[c])
        in_maps[c]["src_tab"] = gs
        in_maps[c]["self_rows"] = np.ascontiguousarray(gs[c * npc:(c + 1) * npc])
        in_maps[c]["w"] = w2
        in_maps[c].pop("bias_full", None)
        if hb2:
            in_maps[c]["bias_full"] = np.ascontiguousarray(
                np.broadcast_to(b2c, (P, cfg.out2_f)).astype(np.float32))
    res2 = runner(nc2, in_maps)

    out = np.zeros((cfg.n_pad, cfg.out2_f), dtype=np.float32)
    for c in range(cfg.n_cores):
        out[c * npc:(c + 1) * npc] = res2[c]["out_t"]
    out = out[:cfg.n_nodes]
    if want_times and times is not None:
        return out, times
    return out


def kernel(x, edge_index, W1, b1, W2, b2):
    return _run_gcn(x, edge_index, W1, b1, W2, b2, FULL)


# revision 22
# speedup vs baseline: 1.2217x; 1.2217x over previous
"""Two-layer GCN (PyG GCNConv x2 + rrelu) on 8 Trainium2 NeuronCores.

Math: with A = adjacency-with-multiplicity + I (self loops), deg = in-degree
(including the self loop), dinv = deg^-1/2:
    z1[v] = dinv[v] * (sum_{u->v} dinv[u]*x[u]) @ W1 + b1
    g[u]  = dinv[u] * rrelu(z1[u])                      (dinv pre-folded for L2)
    z2[v] = dinv[v] * (sum_{u->v} g[u]) @ W2 + b2
Aggregation is linear, so the dense W matmul is applied post-aggregation on
the [128, 128] per-destination-block aggregate -- 128x less PE work than
transforming every edge message.

Sharding: destinations are range-sharded across the 8 cores (12544 each).
Every core keeps a replicated (dinv-prescaled, bf16) source-feature table in
its own HBM and fetches the source rows of its edges with dma_gather (int16
indices -> four 25088-row source windows).  The SWDGE descriptor-generation
fixed cost (~1us/call) dominated the previous version (one call per
(block, window) = 392/layer), so gathers are batched to ONE call per
(superblock of 7 blocks, window) = 56/layer with a window-major msg layout
so each call writes contiguous columns.

Per destination block of 128 nodes, gathered edge-message chunks
[128 edges, 128 feat] are scatter-reduced on the TensorEngine by matmul with
one-hot selectors Sel[e, dest] = (d[e] == dest) generated on-device
(is_equal with broadcast operand).  Self-loop contributions bypass the
gather: their source rows are contiguous, so a plain DMA + identity matmul
adds them.  The dense W matmul uses the f32->bf16-cast aggregate as the
STATIONARY operand so z comes out dest-major [dest, outF]; dinv[dest] is
then a per-partition scalar, letting the otherwise idle Scalar engine apply
the dinv scaling and rrelu (Prelu activation) off the Vector engine's
critical path.  Outputs are written dest-major = node-major, so the host
never transposes.  Two NEFF dispatches (layer 1, layer 2).

The harness calls kernel(**inputs) with full inputs; index bucketing,
program build, compile, SPMD run on cores 0-7 and unshard all happen here.
"""

import sys

for _p in ("/opt/trn_rl_repo",):
    if _p not in sys.path:
        sys.path.insert(0, _p)

import numpy as np
import ml_dtypes

import concourse.bacc as bacc
import concourse.bass as bass
import concourse.mybir as mybir
import concourse.tile as tile
from concourse.bass_utils import run_bass_kernel_spmd

P = 128  # partition width == dest block width == feature width
RRELU_SLOPE = (1.0 / 8.0 + 1.0 / 3.0) / 2.0
MAX_CALL_COLS = 7   # dma_gather is capped at 1008 indices per call


def _call_plan(win_cols, ovf_start=None):
    """Per-superblock gather calls [(window, col0, ncols)], chunked to
    MAX_CALL_COLS and round-robin interleaved across windows so the four
    SWDGE queues fill evenly (a queue's ring holds only ~2 calls; emitting
    one window's calls back-to-back blocks GpSimd and starves the rest).
    The shared overflow cols (from ovf_start[k] on) are emitted FIRST:
    every block reads them, so late arrival stalls the whole superblock."""
    plan = []
    if ovf_start is not None:
        for k in range(len(win_cols)):
            c0 = ovf_start[k]
            while c0 < win_cols[k]:
                ncols = min(MAX_CALL_COLS, win_cols[k] - c0)
                plan.append((k, c0, ncols))
                c0 += ncols
        win_cols = list(ovf_start)
    pending = [(k, 0, win_cols[k]) for k in range(len(win_cols))]
    while pending:
        nxt = []
        for (k, c0, total) in pending:
            ncols = min(MAX_CALL_COLS, total - c0)
            plan.append((k, c0, ncols))
            if c0 + ncols < total:
                nxt.append((k, c0 + ncols, total))
        pending = nxt
    return plan


class Cfg:
    def __init__(self, n_nodes, n_cores, blocks_per_core, superblock, in_f,
                 out1_f, out2_f, src_window, dense_cols=4):
        self.n_nodes = n_nodes
        self.n_cores = n_cores
        self.bpc = blocks_per_core            # dest blocks per core
        self.sb = superblock                  # blocks per superblock
        assert blocks_per_core % superblock == 0
        self.sb_count = blocks_per_core // superblock
        self.in_f = in_f
        self.out1_f = out1_f
        self.out2_f = out2_f
        self.src_window = src_window          # int16 gather range per window
        self.dense_cols = dense_cols          # private cols per (block, win)
        self.nodes_per_core = blocks_per_core * P
        self.n_pad = n_cores * self.nodes_per_core
        assert self.n_pad >= n_nodes
        assert src_window % P == 0 and src_window <= 32768
        self.n_chunks = -(-self.n_pad // src_window)
        self.tab_rows = self.n_chunks * src_window


FULL = Cfg(n_nodes=100000, n_cores=8, blocks_per_core=98, superblock=7,
           in_f=128, out1_f=128, out2_f=64, src_window=25088, dense_cols=4)


# --------------------------------------------------------------------------
# host-side index preprocessing
# --------------------------------------------------------------------------

def preprocess(edge_index, cfg):
    """Bucket edges by (dest block, src window); self loops are handled
    separately on-device.  Build per-core gather index / dest-local tables
    and the degree scaling.

    Column layout (per superblock, window-major):
      [win k: sb*DENSE dense cols | ovf[k] overflow cols]  for k in windows
    Each (block, window) gets DENSE=4 private columns (512 slots); edges
    beyond 512 spill into the window's shared overflow columns, which are
    scatter-matmul'ed once per block with a per-block d_tab column (non-own
    edges masked to -1).  This cuts gather padding from ~20% to a few %
    while keeping PE/DVE work per block unchanged."""
    row = edge_index[0].astype(np.int64)
    col = edge_index[1].astype(np.int64)
    n = cfg.n_nodes

    deg = np.bincount(col, minlength=cfg.n_pad).astype(np.float64) + 1.0
    dinv = (1.0 / np.sqrt(deg)).astype(np.float32)
    dinv[n:] = 1.0

    blk = col >> 7                      # global dest block
    chunk = row // cfg.src_window
    order = np.lexsort((chunk, blk))
    row, col, blk, chunk = row[order], col[order], blk[order], chunk[order]

    n_blocks = cfg.n_cores * cfg.bpc
    counts = np.zeros((n_blocks, cfg.n_chunks), dtype=np.int64)
    np.add.at(counts, (blk, chunk), 1)

    bc_start = np.zeros(n_blocks * cfg.n_chunks + 1, dtype=np.int64)
    np.cumsum(counts.reshape(-1), out=bc_start[1:])

    DENSE = cfg.dense_cols
    dn = DENSE * P
    # overflow slots per (core, superblock, window) -> uniform col caps
    ovf_cnt = np.zeros((cfg.n_cores, cfg.sb_count, cfg.n_chunks), dtype=np.int64)
    for c in range(cfg.n_cores):
        for s in range(cfg.sb_count):
            for k in range(cfg.n_chunks):
                tot = 0
                for b7 in range(cfg.sb):
                    b_glob = c * cfg.bpc + s * cfg.sb + b7
                    tot += max(0, counts[b_glob, k] - dn)
                ovf_cnt[c, s, k] = tot
    ovf_cols = [int(-(-int(ovf_cnt[:, :, k].max()) // P)) for k in range(cfg.n_chunks)]
    c_blk = cfg.n_chunks * DENSE + sum(ovf_cols)   # d_tab cols per block
    win_cols = [cfg.sb * DENSE + ovf_cols[k] for k in range(cfg.n_chunks)]
    wbase = np.concatenate([[0], np.cumsum(win_cols)]).astype(int)
    sb_cols = int(wbase[-1])                       # msg cols per superblock

    per_core = []
    for c in range(cfg.n_cores):
        d_tab = np.full((P, cfg.bpc * c_blk), -1.0, dtype=np.float64)
        # per (s, k): dense idx [sb*dn], ovf idx [ovf_cols[k]*P]
        dense_idx = np.zeros((cfg.sb_count, cfg.n_chunks, cfg.sb * dn), np.int64)
        ovf_idx = [np.zeros((cfg.sb_count, ovf_cols[k] * P), np.int64)
                   for k in range(cfg.n_chunks)]
        for s in range(cfg.sb_count):
            for k in range(cfg.n_chunks):
                ov_rows, ov_dest, ov_blk7 = [], [], []
                for b7 in range(cfg.sb):
                    b_loc = s * cfg.sb + b7
                    b_glob = c * cfg.bpc + b_loc
                    lo = bc_start[b_glob * cfg.n_chunks + k]
                    hi = bc_start[b_glob * cfg.n_chunks + k + 1]
                    r_all = row[lo:hi] - k * cfg.src_window
                    d_all = col[lo:hi] - b_glob * P
                    nd = min(len(r_all), dn)
                    seg = np.zeros(dn, dtype=np.int64)
                    seg[:nd] = r_all[:nd]
                    if nd < dn:
                        seg[nd:] = seg[0] if nd > 0 else 0
                    dense_idx[s, k, b7 * dn:(b7 + 1) * dn] = seg
                    d_seg = np.full(dn, -1.0)
                    d_seg[:nd] = d_all[:nd].astype(np.float64)
                    gcol0 = b_loc * c_blk + k * DENSE
                    d_tab[:, gcol0:gcol0 + DENSE] = d_seg.reshape(DENSE, P).T
                    if len(r_all) > dn:
                        ov_rows.append(r_all[dn:])
                        ov_dest.append(d_all[dn:])
                        ov_blk7.append(np.full(len(r_all) - dn, b7))
                vcols = ovf_cols[k]
                if vcols == 0:
                    continue
                vslots = vcols * P
                if ov_rows:
                    orow = np.concatenate(ov_rows)
                    odst = np.concatenate(ov_dest)
                    ob7 = np.concatenate(ov_blk7)
                else:
                    orow = np.zeros(0, np.int64)
                    odst = np.zeros(0, np.int64)
                    ob7 = np.zeros(0, np.int64)
                cnt = len(orow)
                assert cnt <= vslots, (cnt, vslots)
                oseg = np.zeros(vslots, dtype=np.int64)
                oseg[:cnt] = orow
                if cnt < vslots:
                    oseg[cnt:] = oseg[0] if cnt > 0 else 0
                ovf_idx[k][s] = oseg
                # per-block d columns for the shared overflow cols
                for b7 in range(cfg.sb):
                    b_loc = s * cfg.sb + b7
                    dv = np.full(vslots, -1.0)
                    mine = ob7 == b7
                    dv[:cnt][mine] = odst[mine].astype(np.float64)
                    gcol0 = b_loc * c_blk + cfg.n_chunks * DENSE + int(np.sum(ovf_cols[:k]))
                    d_tab[:, gcol0:gcol0 + vcols] = dv.reshape(vcols, P).T
        # idx table in call-emission order (round-robin interleaved over
        # windows so all SWDGE queues stay fed -- see _call_plan)
        plan = _call_plan(win_cols, [cfg.sb * DENSE] * cfg.n_chunks)
        idx_parts = []
        for s in range(cfg.sb_count):
            win_flat = [np.concatenate([dense_idx[s, k],
                                        ovf_idx[k][s]])
                        for k in range(cfg.n_chunks)]
            for (k, c0, ncols) in plan:
                idx_parts.append(
                    win_flat[k][c0 * P:(c0 + ncols) * P].astype(np.int16))
        idx_flat = [a.reshape(-1, 16).T for a in idx_parts]
        idx_tab = np.concatenate(idx_flat, axis=1)
        idx_tab = np.tile(idx_tab, (8, 1))          # [128, total/16]
        # dinv columns: dinv_cols[p, b] = dinv[core_base + b*128 + p]
        dslice = dinv[c * cfg.nodes_per_core:(c + 1) * cfg.nodes_per_core]
        dinv_cols = np.ascontiguousarray(dslice.reshape(cfg.bpc, P).T)
        per_core.append({
            "idx_tab": np.ascontiguousarray(idx_tab),
            "d_tab": np.ascontiguousarray(d_tab.astype(ml_dtypes.bfloat16)),
            "dinv_cols": dinv_cols,
        })

    return {"ovf_cols": ovf_cols, "c_blk": c_blk, "dinv": dinv,
            "per_core": per_core}


# --------------------------------------------------------------------------
# bass program (one GCN layer, SPMD across cores; all data via inputs)
# --------------------------------------------------------------------------

def build_layer_program(cfg, ovf_cols, layer, has_bias=False):
    """layer=1: out = bf16 g [nodes_per_core, 128]  (dinv*rrelu(z1), node-major)
       layer=2: out = f32  z2 [nodes_per_core, out2_f]"""
    DENSE = cfg.dense_cols
    ovf_cols = [int(x) for x in ovf_cols]
    c_blk = cfg.n_chunks * DENSE + sum(ovf_cols)
    win_cols = [cfg.sb * DENSE + ovf_cols[k] for k in range(cfg.n_chunks)]
    wbase = [0]
    for wc in win_cols:
        wbase.append(wbase[-1] + wc)
    sb_cols = wbase[-1]                      # msg cols per superblock
    plan = _call_plan(win_cols, [cfg.sb * DENSE] * cfg.n_chunks)
    # per block: (msg col, d_tab col-within-block) in enumeration order
    def block_cols(b7):
        cols = []
        for k in range(cfg.n_chunks):
            for cd in range(DENSE):
                cols.append((wbase[k] + b7 * DENSE + cd, k * DENSE + cd))
        for k in range(cfg.n_chunks):
            for v in range(ovf_cols[k]):
                cols.append((wbase[k] + cfg.sb * DENSE + v,
                             cfg.n_chunks * DENSE + sum(ovf_cols[:k]) + v))
        return cols
    out_f = cfg.out1_f if layer == 1 else cfg.out2_f
    out_dt = mybir.dt.bfloat16 if layer == 1 else mybir.dt.float32
    idx_cols_sb = sb_cols * P // 16          # idx free-dim per superblock
    G = 8                                     # sel-gen chunk group width

    nc = bacc.Bacc("TRN2", target_bir_lowering=False, debug=False,
                   num_devices=cfg.n_cores,
                   num_swdge_queues=min(4, cfg.n_chunks))
    dt = mybir.dt
    src_tab = nc.dram_tensor("src_tab", [cfg.tab_rows, P], dt.bfloat16,
                             kind="ExternalInput")
    w_in = nc.dram_tensor("w", [P, out_f], dt.bfloat16, kind="ExternalInput")
    dinv_in = nc.dram_tensor("dinv_cols", [P, cfg.bpc], dt.float32,
                             kind="ExternalInput")
    idx_in = nc.dram_tensor("idx_tab", [P, cfg.sb_count * idx_cols_sb], dt.int16,
                            kind="ExternalInput")
    d_in = nc.dram_tensor("d_tab", [P, cfg.bpc * c_blk], dt.bfloat16,
                          kind="ExternalInput")
    iota_in = nc.dram_tensor("iota", [P, G * P], dt.bfloat16, kind="ExternalInput")
    ident_in = nc.dram_tensor("ident", [P, P], dt.bfloat16, kind="ExternalInput")
    out_t = nc.dram_tensor("out_t", [cfg.nodes_per_core, out_f], out_dt,
                           kind="ExternalOutput")
    # per-core self-loop source rows, staged by the host (node-major slice of
    # src_tab rows owned by this core; avoids needing the core id on device)
    self_in = nc.dram_tensor("self_rows", [cfg.nodes_per_core, P], dt.bfloat16,
                             kind="ExternalInput")
    if has_bias:
        bias_in = nc.dram_tensor("bias_full", [P, out_f], dt.float32,
                                 kind="ExternalInput")

    with tile.TileContext(nc) as tc:
        with (
            tc.tile_pool(name="const", bufs=1) as const_pool,
            tc.tile_pool(name="idx", bufs=3) as idx_pool,
            tc.tile_pool(name="msg", bufs=3) as msg_pool,
            tc.tile_pool(name="selfp", bufs=3) as self_pool,
            tc.tile_pool(name="sel", bufs=6) as sel_pool,
            tc.tile_pool(name="aggsb", bufs=3) as aggsb_pool,
            tc.tile_pool(name="tmp", bufs=3) as tmp_pool,
            tc.tile_pool(name="outsb", bufs=2) as out_pool,
            tc.tile_pool(name="psA", bufs=3, space="PSUM") as agg_psum,
            tc.tile_pool(name="psZ", bufs=2, space="PSUM") as z_psum,
        ):
            w_sb = const_pool.tile([P, out_f], dt.bfloat16)
            nc.sync.dma_start(out=w_sb[:], in_=w_in[:])
            dinv_sb = const_pool.tile([P, cfg.bpc], dt.float32)
            nc.sync.dma_start(out=dinv_sb[:], in_=dinv_in[:])
            iota_sb = const_pool.tile([P, G * P], dt.bfloat16)
            nc.sync.dma_start(out=iota_sb[:], in_=iota_in[:])
            ident_sb = const_pool.tile([P, P], dt.bfloat16)
            nc.sync.dma_start(out=ident_sb[:], in_=ident_in[:])
            d_sb = const_pool.tile([P, cfg.bpc * c_blk], dt.bfloat16)
            nc.sync.dma_start(out=d_sb[:], in_=d_in[:])
            alpha_sb = const_pool.tile([P, 1], dt.float32)
            nc.vector.memset(alpha_sb[:], float(RRELU_SLOPE))
            if has_bias:
                bias_sb = const_pool.tile([P, out_f], dt.float32)
                nc.sync.dma_start(out=bias_sb[:], in_=bias_in[:])

            self_view = self_in.rearrange("(s b p) f -> s p b f",
                                          p=P, b=cfg.sb)
            out_view = out_t.rearrange("(s b p) f -> s p b f",
                                       p=P, b=cfg.sb)

            def finish_block(b_loc, aggsb, out_sb, b7):
                """W matmul (agg stationary -> z dest-major) + ACT epilogue."""
                zps = z_psum.tile([P, out_f], dt.float32)
                nc.tensor.matmul(zps[:], lhsT=aggsb[:], rhs=w_sb[:],
                                 start=True, stop=True)
                dcol = dinv_sb[:, b_loc:b_loc + 1]
                o_sl = out_sb[:, b7, :]
                if layer == 1:
                    t1 = tmp_pool.tile([P, out_f], dt.float32, tag="t1")
                    if has_bias:
                        tz = tmp_pool.tile([P, out_f], dt.float32, tag="tz")
                        nc.scalar.activation(
                            tz[:], zps[:], mybir.ActivationFunctionType.Copy,
                            scale=dcol)
                        tb = tmp_pool.tile([P, out_f], dt.float32, tag="tb")
                        nc.vector.tensor_tensor(tb[:], tz[:], bias_sb[:],
                                                mybir.AluOpType.add)
                        nc.scalar.activation(
                            t1[:], tb[:], mybir.ActivationFunctionType.Prelu,
                            scale=1.0, alpha=alpha_sb[:, 0:1])
                    else:
                        nc.scalar.activation(
                            t1[:], zps[:], mybir.ActivationFunctionType.Prelu,
                            scale=dcol, alpha=alpha_sb[:, 0:1])
                    nc.scalar.activation(
                        o_sl, t1[:], mybir.ActivationFunctionType.Copy,
                        scale=dcol)
                else:
                    if has_bias:
                        tz = tmp_pool.tile([P, out_f], dt.float32, tag="tz")
                        nc.scalar.activation(
                            tz[:], zps[:], mybir.ActivationFunctionType.Copy,
                            scale=dcol)
                        nc.vector.tensor_tensor(o_sl, tz[:], bias_sb[:],
                                                mybir.AluOpType.add)
                    else:
                        nc.scalar.activation(
                            o_sl, zps[:], mybir.ActivationFunctionType.Copy,
                            scale=dcol)

            for s in range(cfg.sb_count):
                idx_sb = idx_pool.tile([P, idx_cols_sb], dt.int16)
                nc.sync.dma_start(
                    out=idx_sb[:],
                    in_=idx_in[:, s * idx_cols_sb:(s + 1) * idx_cols_sb])
                # contiguous self-loop rows for this superblock
                selfs = self_pool.tile([P, cfg.sb, P], dt.bfloat16)
                nc.sync.dma_start(out=selfs[:], in_=self_view[s])

                # gather calls per window (window-major msg cols), chunked to
                # MAX_CALL_COLS columns and interleaved across queues
                msg = msg_pool.tile([P, sb_cols, P], dt.bfloat16)
                off = 0
                for (k, c0, ncols) in plan:
                    mcol0 = wbase[k] + c0
                    n_idx = ncols * P
                    nc.gpsimd.dma_gather(
                        msg[:, mcol0:mcol0 + ncols, :],
                        src_tab[k * cfg.src_window:
                                (k + 1) * cfg.src_window, :],
                        idx_sb[:, off:off + n_idx // 16],
                        n_idx, n_idx, P,
                        queue_num=k % 4,
                    )
                    off += n_idx // 16

                out_sb = out_pool.tile([P, cfg.sb, out_f], out_dt)
                pending = None  # (b_loc, aggsb, b7) 1-deep pipeline
                for b7 in range(cfg.sb):
                    b_loc = s * cfg.sb + b7
                    dcol0 = b_loc * c_blk
                    cols = block_cols(b7)
                    sels = []
                    done = 0
                    while done < c_blk:
                        g = min(G, c_blk - done)
                        sel = sel_pool.tile([P, G * P], dt.bfloat16)
                        nc.vector.tensor_tensor(
                            sel[:, :g * P],
                            iota_sb[:, :g * P],
                            d_sb[:, dcol0 + done:dcol0 + done + g]
                                .to_broadcast([P, g, P]),
                            mybir.AluOpType.is_equal,
                        )
                        sels.extend((sel, j) for j in range(g))
                        done += g

                    agg = agg_psum.tile([P, P], dt.float32)
                    for ci, (sel, j) in enumerate(sels):
                        mcol = cols[ci][0]
                        nc.tensor.matmul(
                            agg[:],
                            lhsT=msg[:, mcol, :],
                            rhs=sel[:, j * P:(j + 1) * P],
                            start=(ci == 0), stop=False,
                        )
                    # self-loop contribution: aggT += selfs[:, b7, :]^T
                    nc.tensor.matmul(
                        agg[:], lhsT=selfs[:, b7, :], rhs=ident_sb[:],
                        start=False, stop=True)

                    aggsb = aggsb_pool.tile([P, P], dt.bfloat16)
                    nc.vector.tensor_copy(aggsb[:], agg[:])

                    if pending is not None:
                        finish_block(*pending)
                    pending = (b_loc, aggsb, out_sb, b7)
                finish_block(*pending)

                nc.sync.dma_start(out=out_view[s], in_=out_sb[:])

    nc.compile()
    return nc


# --------------------------------------------------------------------------
# orchestration
# --------------------------------------------------------------------------

def _iota_tile(G=8):
    return np.tile(np.arange(P, dtype=np.float32), G)[None, :].repeat(P, 0).astype(ml_dtypes.bfloat16)


def _run_gcn(x, edge_index, W1, b1, W2, b2, cfg, runner=None, want_times=False):
    """Shared driver; runner(nc, in_maps) -> list of per-core output dicts."""
    meta = preprocess(np.asarray(edge_index), cfg)
    dinv = meta["dinv"]
    npc = cfg.nodes_per_core

    if runner is None:
        times = []

        def runner(nc, in_maps):
            r = run_bass_kernel_spmd(nc, in_maps, core_ids=list(range(cfg.n_cores)),
                                     trace=want_times)
            if want_times:
                times.append(r.exec_time_ns)
            return r.results
    else:
        times = None

    x = np.asarray(x, dtype=np.float32)
    xs = np.zeros((cfg.tab_rows, P), dtype=ml_dtypes.bfloat16)
    xs[:cfg.n_nodes] = (x * dinv[:cfg.n_nodes, None]).astype(ml_dtypes.bfloat16)

    iota = _iota_tile()
    ident = np.eye(P, dtype=np.float32).astype(ml_dtypes.bfloat16)
    w1 = np.asarray(W1, np.float32).astype(ml_dtypes.bfloat16)
    w2 = np.asarray(W2, np.float32).astype(ml_dtypes.bfloat16)
    b1c = np.asarray(b1, np.float32).reshape(-1)
    b2c = np.asarray(b2, np.float32).reshape(-1)
    hb1 = bool(np.any(b1c != 0.0))
    hb2 = bool(np.any(b2c != 0.0))

    nc1 = build_layer_program(cfg, meta["ovf_cols"], layer=1, has_bias=hb1)
    in_maps = [
        {"src_tab": xs, "w": w1, "iota": iota, "ident": ident,
         "self_rows": np.ascontiguousarray(xs[c * npc:(c + 1) * npc]),
         **{k: pc[k] for k in ("idx_tab", "d_tab", "dinv_cols")}}
        for c, pc in enumerate(meta["per_core"])
    ]
    if hb1:
        bf = np.ascontiguousarray(np.broadcast_to(b1c, (P, cfg.out1_f)).astype(np.float32))
        for m in in_maps:
            m["bias_full"] = bf
    res1 = runner(nc1, in_maps)

    gs = np.zeros((cfg.tab_rows, P), dtype=ml_dtypes.bfloat16)
    for c in range(cfg.n_cores):
        gs[c * npc:(c + 1) * npc] = res1[c]["out_t"]

    nc2 = build_layer_program(cfg, meta["ovf_cols"], layer=2, has_bias=hb2)
    for c in range(cfg.n_cores):
        in_maps[c] = dict(in_maps[c])
        in_maps[c]["src_tab"] = gs
        in_maps[c]["self_rows"] = np.ascontiguousarray(gs[c * npc:(c + 1) * npc])
        in_maps[c]["w"] = w2
        in_maps[c].pop("bias_full", None)
        if hb2:
            in_maps[c]["bias_full"] = np.ascontiguousarray(
                np.broadcast_to(b2c, (P, cfg.out2_f)).astype(np.float32))
    res2 = runner(nc2, in_maps)

    out = np.zeros((cfg.n_pad, cfg.out2_f), dtype=np.float32)
    for c in range(cfg.n_cores):
        out[c * npc:(c + 1) * npc] = res2[c]["out_t"]
    out = out[:cfg.n_nodes]
    if want_times and times is not None:
        return out, times
    return out


def kernel(x, edge_index, W1, b1, W2, b2):
    return _run_gcn(x, edge_index, W1, b1, W2, b2, FULL)


# revision 24
# speedup vs baseline: 1.2277x; 1.0049x over previous
"""Two-layer GCN (PyG GCNConv x2 + rrelu) on 8 Trainium2 NeuronCores.

Math: with A = adjacency-with-multiplicity + I (self loops), deg = in-degree
(including the self loop), dinv = deg^-1/2:
    z1[v] = dinv[v] * (sum_{u->v} dinv[u]*x[u]) @ W1 + b1
    g[u]  = dinv[u] * rrelu(z1[u])                      (dinv pre-folded for L2)
    z2[v] = dinv[v] * (sum_{u->v} g[u]) @ W2 + b2
Aggregation is linear, so the dense W matmul is applied post-aggregation on
the [128, 128] per-destination-block aggregate -- 128x less PE work than
transforming every edge message.

Sharding: destinations are range-sharded across the 8 cores (12544 each).
Every core keeps a replicated (dinv-prescaled, bf16) source-feature table in
its own HBM and fetches the source rows of its edges with dma_gather (int16
indices -> four 25088-row source windows).  The SWDGE descriptor-generation
fixed cost (~1us/call) dominated the previous version (one call per
(block, window) = 392/layer), so gathers are batched to ONE call per
(superblock of 7 blocks, window) = 56/layer with a window-major msg layout
so each call writes contiguous columns.

Per destination block of 128 nodes, gathered edge-message chunks
[128 edges, 128 feat] are scatter-reduced on the TensorEngine by matmul with
one-hot selectors Sel[e, dest] = (d[e] == dest) generated on-device
(is_equal with broadcast operand).  Self-loop contributions bypass the
gather: their source rows are contiguous, so a plain DMA + identity matmul
adds them.  The dense W matmul uses the f32->bf16-cast aggregate as the
STATIONARY operand so z comes out dest-major [dest, outF]; dinv[dest] is
then a per-partition scalar, letting the otherwise idle Scalar engine apply
the dinv scaling and rrelu (Prelu activation) off the Vector engine's
critical path.  Outputs are written dest-major = node-major, so the host
never transposes.  Two NEFF dispatches (layer 1, layer 2).

The harness calls kernel(**inputs) with full inputs; index bucketing,
program build, compile, SPMD run on cores 0-7 and unshard all happen here.
"""

import sys

for _p in ("/opt/trn_rl_repo",):
    if _p not in sys.path:
        sys.path.insert(0, _p)

import numpy as np
import ml_dtypes

import concourse.bacc as bacc
import concourse.bass as bass
import concourse.mybir as mybir
import concourse.tile as tile
from concourse.bass_utils import run_bass_kernel_spmd

P = 128  # partition width == dest block width == feature width
RRELU_SLOPE = (1.0 / 8.0 + 1.0 / 3.0) / 2.0
MAX_CALL_COLS = 7   # dma_gather is capped at 1008 indices per call


def _call_plan(win_cols, ovf_start=None):
    """Per-superblock gather calls [(window, col0, ncols)], chunked to
    MAX_CALL_COLS and round-robin interleaved across windows so the four
    SWDGE queues fill evenly (a queue's ring holds only ~2 calls; emitting
    one window's calls back-to-back blocks GpSimd and starves the rest).
    The shared overflow cols (from ovf_start[k] on) are emitted FIRST:
    every block reads them, so late arrival stalls the whole superblock."""
    plan = []
    if ovf_start is not None:
        for k in range(len(win_cols)):
            c0 = ovf_start[k]
            while c0 < win_cols[k]:
                ncols = min(MAX_CALL_COLS, win_cols[k] - c0)
                plan.append((k, c0, ncols))
                c0 += ncols
        win_cols = list(ovf_start)
    pending = [(k, 0, win_cols[k]) for k in range(len(win_cols))]
    while pending:
        nxt = []
        for (k, c0, total) in pending:
            ncols = min(MAX_CALL_COLS, total - c0)
            plan.append((k, c0, ncols))
            if c0 + ncols < total:
                nxt.append((k, c0 + ncols, total))
        pending = nxt
    return plan


class Cfg:
    def __init__(self, n_nodes, n_cores, blocks_per_core, superblock, in_f,
                 out1_f, out2_f, src_window, dense_cols=4):
        self.n_nodes = n_nodes
        self.n_cores = n_cores
        self.bpc = blocks_per_core            # dest blocks per core
        self.sb = superblock                  # blocks per superblock
        assert blocks_per_core % superblock == 0
        self.sb_count = blocks_per_core // superblock
        self.in_f = in_f
        self.out1_f = out1_f
        self.out2_f = out2_f
        self.src_window = src_window          # int16 gather range per window
        self.dense_cols = dense_cols          # private cols per (block, win)
        self.nodes_per_core = blocks_per_core * P
        self.n_pad = n_cores * self.nodes_per_core
        assert self.n_pad >= n_nodes
        assert src_window % P == 0 and src_window <= 32768
        self.n_chunks = -(-self.n_pad // src_window)
        self.tab_rows = self.n_chunks * src_window


FULL = Cfg(n_nodes=100000, n_cores=8, blocks_per_core=98, superblock=7,
           in_f=128, out1_f=128, out2_f=64, src_window=25088, dense_cols=4)


# --------------------------------------------------------------------------
# host-side index preprocessing
# --------------------------------------------------------------------------

def preprocess(edge_index, cfg):
    """Bucket edges by (dest block, src window); self loops are handled
    separately on-device.  Build per-core gather index / dest-local tables
    and the degree scaling.

    Column layout (per superblock, window-major):
      [win k: sb*DENSE dense cols | ovf[k] overflow cols]  for k in windows
    Each (block, window) gets DENSE=4 private columns (512 slots); edges
    beyond 512 spill into the window's shared overflow columns, which are
    scatter-matmul'ed once per block with a per-block d_tab column (non-own
    edges masked to -1).  This cuts gather padding from ~20% to a few %
    while keeping PE/DVE work per block unchanged."""
    row = edge_index[0].astype(np.int64)
    col = edge_index[1].astype(np.int64)
    n = cfg.n_nodes

    deg = np.bincount(col, minlength=cfg.n_pad).astype(np.float64) + 1.0
    dinv = (1.0 / np.sqrt(deg)).astype(np.float32)
    dinv[n:] = 1.0

    blk = col >> 7                      # global dest block
    chunk = row // cfg.src_window
    order = np.lexsort((chunk, blk))
    row, col, blk, chunk = row[order], col[order], blk[order], chunk[order]

    n_blocks = cfg.n_cores * cfg.bpc
    counts = np.zeros((n_blocks, cfg.n_chunks), dtype=np.int64)
    np.add.at(counts, (blk, chunk), 1)

    bc_start = np.zeros(n_blocks * cfg.n_chunks + 1, dtype=np.int64)
    np.cumsum(counts.reshape(-1), out=bc_start[1:])

    DENSE = cfg.dense_cols
    dn = DENSE * P
    # overflow slots per (core, superblock, window) -> uniform col caps
    ovf_cnt = np.zeros((cfg.n_cores, cfg.sb_count, cfg.n_chunks), dtype=np.int64)
    for c in range(cfg.n_cores):
        for s in range(cfg.sb_count):
            for k in range(cfg.n_chunks):
                tot = 0
                for b7 in range(cfg.sb):
                    b_glob = c * cfg.bpc + s * cfg.sb + b7
                    tot += max(0, counts[b_glob, k] - dn)
                ovf_cnt[c, s, k] = tot
    ovf_cols = [int(-(-int(ovf_cnt[:, :, k].max()) // P)) for k in range(cfg.n_chunks)]
    c_blk = cfg.n_chunks * DENSE + sum(ovf_cols)   # d_tab cols per block
    win_cols = [cfg.sb * DENSE + ovf_cols[k] for k in range(cfg.n_chunks)]
    wbase = np.concatenate([[0], np.cumsum(win_cols)]).astype(int)
    sb_cols = int(wbase[-1])                       # msg cols per superblock

    per_core = []
    for c in range(cfg.n_cores):
        d_tab = np.full((P, cfg.bpc * c_blk), -1.0, dtype=np.float64)
        # per (s, k): dense idx [sb*dn], ovf idx [ovf_cols[k]*P]
        dense_idx = np.zeros((cfg.sb_count, cfg.n_chunks, cfg.sb * dn), np.int64)
        ovf_idx = [np.zeros((cfg.sb_count, ovf_cols[k] * P), np.int64)
                   for k in range(cfg.n_chunks)]
        for s in range(cfg.sb_count):
            for k in range(cfg.n_chunks):
                ov_rows, ov_dest, ov_blk7 = [], [], []
                for b7 in range(cfg.sb):
                    b_loc = s * cfg.sb + b7
                    b_glob = c * cfg.bpc + b_loc
                    lo = bc_start[b_glob * cfg.n_chunks + k]
                    hi = bc_start[b_glob * cfg.n_chunks + k + 1]
                    r_all = row[lo:hi] - k * cfg.src_window
                    d_all = col[lo:hi] - b_glob * P
                    nd = min(len(r_all), dn)
                    seg = np.zeros(dn, dtype=np.int64)
                    seg[:nd] = r_all[:nd]
                    if nd < dn:
                        seg[nd:] = seg[0] if nd > 0 else 0
                    dense_idx[s, k, b7 * dn:(b7 + 1) * dn] = seg
                    d_seg = np.full(dn, -1.0)
                    d_seg[:nd] = d_all[:nd].astype(np.float64)
                    gcol0 = b_loc * c_blk + k * DENSE
                    d_tab[:, gcol0:gcol0 + DENSE] = d_seg.reshape(DENSE, P).T
                    if len(r_all) > dn:
                        ov_rows.append(r_all[dn:])
                        ov_dest.append(d_all[dn:])
                        ov_blk7.append(np.full(len(r_all) - dn, b7))
                vcols = ovf_cols[k]
                if vcols == 0:
                    continue
                vslots = vcols * P
                if ov_rows:
                    orow = np.concatenate(ov_rows)
                    odst = np.concatenate(ov_dest)
                    ob7 = np.concatenate(ov_blk7)
                else:
                    orow = np.zeros(0, np.int64)
                    odst = np.zeros(0, np.int64)
                    ob7 = np.zeros(0, np.int64)
                cnt = len(orow)
                assert cnt <= vslots, (cnt, vslots)
                oseg = np.zeros(vslots, dtype=np.int64)
                oseg[:cnt] = orow
                if cnt < vslots:
                    oseg[cnt:] = oseg[0] if cnt > 0 else 0
                ovf_idx[k][s] = oseg
                # per-block d columns for the shared overflow cols
                for b7 in range(cfg.sb):
                    b_loc = s * cfg.sb + b7
                    dv = np.full(vslots, -1.0)
                    mine = ob7 == b7
                    dv[:cnt][mine] = odst[mine].astype(np.float64)
                    gcol0 = b_loc * c_blk + cfg.n_chunks * DENSE + int(np.sum(ovf_cols[:k]))
                    d_tab[:, gcol0:gcol0 + vcols] = dv.reshape(vcols, P).T
        # idx table in call-emission order (round-robin interleaved over
        # windows so all SWDGE queues stay fed -- see _call_plan)
        plan = _call_plan(win_cols, [cfg.sb * DENSE] * cfg.n_chunks)
        idx_parts = []
        for s in range(cfg.sb_count):
            win_flat = [np.concatenate([dense_idx[s, k],
                                        ovf_idx[k][s]])
                        for k in range(cfg.n_chunks)]
            for (k, c0, ncols) in plan:
                idx_parts.append(
                    win_flat[k][c0 * P:(c0 + ncols) * P].astype(np.int16))
        idx_flat = [a.reshape(-1, 16).T for a in idx_parts]
        idx_tab = np.concatenate(idx_flat, axis=1)
        idx_tab = np.tile(idx_tab, (8, 1))          # [128, total/16]
        # dinv columns: dinv_cols[p, b] = dinv[core_base + b*128 + p]
        dslice = dinv[c * cfg.nodes_per_core:(c + 1) * cfg.nodes_per_core]
        dinv_cols = np.ascontiguousarray(dslice.reshape(cfg.bpc, P).T)
        per_core.append({
            "idx_tab": np.ascontiguousarray(idx_tab),
            "d_tab": np.ascontiguousarray(d_tab.astype(ml_dtypes.bfloat16)),
            "dinv_cols": dinv_cols,
        })

    return {"ovf_cols": ovf_cols, "c_blk": c_blk, "dinv": dinv,
            "per_core": per_core}


# --------------------------------------------------------------------------
# bass program (one GCN layer, SPMD across cores; all data via inputs)
# --------------------------------------------------------------------------

def build_layer_program(cfg, ovf_cols, layer, has_bias=False):
    """layer=1: out = bf16 g [nodes_per_core, 128]  (dinv*rrelu(z1), node-major)
       layer=2: out = f32  z2 [nodes_per_core, out2_f]"""
    DENSE = cfg.dense_cols
    ovf_cols = [int(x) for x in ovf_cols]
    c_blk = cfg.n_chunks * DENSE + sum(ovf_cols)
    win_cols = [cfg.sb * DENSE + ovf_cols[k] for k in range(cfg.n_chunks)]
    wbase = [0]
    for wc in win_cols:
        wbase.append(wbase[-1] + wc)
    sb_cols = wbase[-1]                      # msg cols per superblock
    plan = _call_plan(win_cols, [cfg.sb * DENSE] * cfg.n_chunks)
    # per block: (msg col, d_tab col-within-block) in enumeration order
    def block_cols(b7):
        cols = []
        for k in range(cfg.n_chunks):
            for cd in range(DENSE):
                cols.append((wbase[k] + b7 * DENSE + cd, k * DENSE + cd))
        for k in range(cfg.n_chunks):
            for v in range(ovf_cols[k]):
                cols.append((wbase[k] + cfg.sb * DENSE + v,
                             cfg.n_chunks * DENSE + sum(ovf_cols[:k]) + v))
        return cols
    out_f = cfg.out1_f if layer == 1 else cfg.out2_f
    out_dt = mybir.dt.bfloat16 if layer == 1 else mybir.dt.float32
    idx_cols_sb = sb_cols * P // 16          # idx free-dim per superblock
    G = 8                                     # sel-gen chunk group width

    nc = bacc.Bacc("TRN2", target_bir_lowering=False, debug=False,
                   num_devices=cfg.n_cores,
                   num_swdge_queues=min(4, cfg.n_chunks))
    dt = mybir.dt
    src_tab = nc.dram_tensor("src_tab", [cfg.tab_rows, P], dt.bfloat16,
                             kind="ExternalInput")
    w_in = nc.dram_tensor("w", [P, out_f], dt.bfloat16, kind="ExternalInput")
    dinv_in = nc.dram_tensor("dinv_cols", [P, cfg.bpc], dt.float32,
                             kind="ExternalInput")
    idx_in = nc.dram_tensor("idx_tab", [P, cfg.sb_count * idx_cols_sb], dt.int16,
                            kind="ExternalInput")
    d_in = nc.dram_tensor("d_tab", [P, cfg.bpc * c_blk], dt.bfloat16,
                          kind="ExternalInput")
    iota_in = nc.dram_tensor("iota", [P, G * P], dt.bfloat16, kind="ExternalInput")
    ident_in = nc.dram_tensor("ident", [P, P], dt.bfloat16, kind="ExternalInput")
    out_t = nc.dram_tensor("out_t", [cfg.nodes_per_core, out_f], out_dt,
                           kind="ExternalOutput")
    # per-core self-loop source rows, staged by the host (node-major slice of
    # src_tab rows owned by this core; avoids needing the core id on device)
    self_in = nc.dram_tensor("self_rows", [cfg.nodes_per_core, P], dt.bfloat16,
                             kind="ExternalInput")
    if has_bias:
        bias_in = nc.dram_tensor("bias_full", [P, out_f], dt.float32,
                                 kind="ExternalInput")

    with tile.TileContext(nc) as tc:
        with (
            tc.tile_pool(name="const", bufs=1) as const_pool,
            tc.tile_pool(name="idx", bufs=3) as idx_pool,
            tc.tile_pool(name="msg", bufs=3) as msg_pool,
            tc.tile_pool(name="selfp", bufs=3) as self_pool,
            tc.tile_pool(name="sel", bufs=6) as sel_pool,
            tc.tile_pool(name="aggsb", bufs=3) as aggsb_pool,
            tc.tile_pool(name="tmp", bufs=3) as tmp_pool,
            tc.tile_pool(name="outsb", bufs=2) as out_pool,
            tc.tile_pool(name="psA", bufs=3, space="PSUM") as agg_psum,
            tc.tile_pool(name="psZ", bufs=2, space="PSUM") as z_psum,
        ):
            # prefetch superblock 0's gather indices ahead of the ~1MB of
            # constant loads so the SWDGE stream starts immediately
            self_view0 = self_in.rearrange("(s b p) f -> s p b f", p=P, b=cfg.sb)
            pre_idx = idx_pool.tile([P, idx_cols_sb], dt.int16)
            nc.sync.dma_start(out=pre_idx[:], in_=idx_in[:, 0:idx_cols_sb])
            pre_selfs = self_pool.tile([P, cfg.sb, P], dt.bfloat16)
            nc.sync.dma_start(out=pre_selfs[:], in_=self_view0[0])

            w_sb = const_pool.tile([P, out_f], dt.bfloat16)
            nc.sync.dma_start(out=w_sb[:], in_=w_in[:])
            dinv_sb = const_pool.tile([P, cfg.bpc], dt.float32)
            nc.sync.dma_start(out=dinv_sb[:], in_=dinv_in[:])
            iota_sb = const_pool.tile([P, G * P], dt.bfloat16)
            nc.sync.dma_start(out=iota_sb[:], in_=iota_in[:])
            ident_sb = const_pool.tile([P, P], dt.bfloat16)
            nc.sync.dma_start(out=ident_sb[:], in_=ident_in[:])
            d_sb = const_pool.tile([P, cfg.bpc * c_blk], dt.bfloat16)
            nc.sync.dma_start(out=d_sb[:], in_=d_in[:])
            alpha_sb = const_pool.tile([P, 1], dt.float32)
            nc.vector.memset(alpha_sb[:], float(RRELU_SLOPE))
            if has_bias:
                bias_sb = const_pool.tile([P, out_f], dt.float32)
                nc.sync.dma_start(out=bias_sb[:], in_=bias_in[:])

            self_view = self_in.rearrange("(s b p) f -> s p b f",
                                          p=P, b=cfg.sb)
            out_view = out_t.rearrange("(s b p) f -> s p b f",
                                       p=P, b=cfg.sb)

            def finish_block(b_loc, aggsb, out_sb, b7):
                """W matmul (agg stationary -> z dest-major) + ACT epilogue."""
                zps = z_psum.tile([P, out_f], dt.float32)
                nc.tensor.matmul(zps[:], lhsT=aggsb[:], rhs=w_sb[:],
                                 start=True, stop=True)
                dcol = dinv_sb[:, b_loc:b_loc + 1]
                o_sl = out_sb[:, b7, :]
                if layer == 1:
                    t1 = tmp_pool.tile([P, out_f], dt.float32, tag="t1")
                    if has_bias:
                        tz = tmp_pool.tile([P, out_f], dt.float32, tag="tz")
                        nc.scalar.activation(
                            tz[:], zps[:], mybir.ActivationFunctionType.Copy,
                            scale=dcol)
                        tb = tmp_pool.tile([P, out_f], dt.float32, tag="tb")
                        nc.vector.tensor_tensor(tb[:], tz[:], bias_sb[:],
                                                mybir.AluOpType.add)
                        nc.scalar.activation(
                            t1[:], tb[:], mybir.ActivationFunctionType.Prelu,
                            scale=1.0, alpha=alpha_sb[:, 0:1])
                    else:
                        nc.scalar.activation(
                            t1[:], zps[:], mybir.ActivationFunctionType.Prelu,
                            scale=dcol, alpha=alpha_sb[:, 0:1])
                    nc.scalar.activation(
                        o_sl, t1[:], mybir.ActivationFunctionType.Copy,
                        scale=dcol)
                else:
                    if has_bias:
                        tz = tmp_pool.tile([P, out_f], dt.float32, tag="tz")
                        nc.scalar.activation(
                            tz[:], zps[:], mybir.ActivationFunctionType.Copy,
                            scale=dcol)
                        nc.vector.tensor_tensor(o_sl, tz[:], bias_sb[:],
                                                mybir.AluOpType.add)
                    else:
                        nc.scalar.activation(
                            o_sl, zps[:], mybir.ActivationFunctionType.Copy,
                            scale=dcol)

            for s in range(cfg.sb_count):
                if s == 0:
                    idx_sb, selfs = pre_idx, pre_selfs
                else:
                    idx_sb = idx_pool.tile([P, idx_cols_sb], dt.int16)
                    nc.sync.dma_start(
                        out=idx_sb[:],
                        in_=idx_in[:, s * idx_cols_sb:(s + 1) * idx_cols_sb])
                    # contiguous self-loop rows for this superblock
                    selfs = self_pool.tile([P, cfg.sb, P], dt.bfloat16)
                    nc.sync.dma_start(out=selfs[:], in_=self_view[s])

                # gather calls per window (window-major msg cols), chunked to
                # MAX_CALL_COLS columns and interleaved across queues
                msg = msg_pool.tile([P, sb_cols, P], dt.bfloat16)
                off = 0
                for (k, c0, ncols) in plan:
                    mcol0 = wbase[k] + c0
                    n_idx = ncols * P
                    nc.gpsimd.dma_gather(
                        msg[:, mcol0:mcol0 + ncols, :],
                        src_tab[k * cfg.src_window:
                                (k + 1) * cfg.src_window, :],
                        idx_sb[:, off:off + n_idx // 16],
                        n_idx, n_idx, P,
                        queue_num=k % 4,
                    )
                    off += n_idx // 16

                out_sb = out_pool.tile([P, cfg.sb, out_f], out_dt)
                pending = None  # (b_loc, aggsb, b7) 1-deep pipeline
                for b7 in range(cfg.sb):
                    b_loc = s * cfg.sb + b7
                    dcol0 = b_loc * c_blk
                    cols = block_cols(b7)
                    sels = []
                    done = 0
                    while done < c_blk:
                        g = min(G, c_blk - done)
                        sel = sel_pool.tile([P, G * P], dt.bfloat16)
                        nc.vector.tensor_tensor(
                            sel[:, :g * P],
                            iota_sb[:, :g * P],
                            d_sb[:, dcol0 + done:dcol0 + done + g]
                                .to_broadcast([P, g, P]),
                            mybir.AluOpType.is_equal,
                        )
                        sels.extend((sel, j) for j in range(g))
                        done += g

                    agg = agg_psum.tile([P, P], dt.float32)
                    for ci, (sel, j) in enumerate(sels):
                        mcol = cols[ci][0]
                        nc.tensor.matmul(
                            agg[:],
                            lhsT=msg[:, mcol, :],
                            rhs=sel[:, j * P:(j + 1) * P],
                            start=(ci == 0), stop=False,
                        )
                    # self-loop contribution: aggT += selfs[:, b7, :]^T
                    nc.tensor.matmul(
                        agg[:], lhsT=selfs[:, b7, :], rhs=ident_sb[:],
                        start=False, stop=True)

                    aggsb = aggsb_pool.tile([P, P], dt.bfloat16)
                    nc.vector.tensor_copy(aggsb[:], agg[:])

                    if pending is not None:
                        finish_block(*pending)
                    pending = (b_loc, aggsb, out_sb, b7)
                finish_block(*pending)

                nc.sync.dma_start(out=out_view[s], in_=out_sb[:])

    nc.compile()
    return nc


# --------------------------------------------------------------------------
# orchestration
# --------------------------------------------------------------------------

def _iota_tile(G=8):
    return np.tile(np.arange(P, dtype=np.float32), G)[None, :].repeat(P, 0).astype(ml_dtypes.bfloat16)


def _run_gcn(x, edge_index, W1, b1, W2, b2, cfg, runner=None, want_times=False):
    """Shared driver; runner(nc, in_maps) -> list of per-core output dicts."""
    meta = preprocess(np.asarray(edge_index), cfg)
    dinv = meta["dinv"]
    npc = cfg.nodes_per_core

    if runner is None:
        times = []

        def runner(nc, in_maps):
            r = run_bass_kernel_spmd(nc, in_maps, core_ids=list(range(cfg.n_cores)),
                                     trace=want_times)
            if want_times:
                times.append(r.exec_time_ns)
            return r.results
    else:
        times = None

    x = np.asarray(x, dtype=np.float32)
    xs = np.zeros((cfg.tab_rows, P), dtype=ml_dtypes.bfloat16)
    xs[:cfg.n_nodes] = (x * dinv[:cfg.n_nodes, None]).astype(ml_dtypes.bfloat16)

    iota = _iota_tile()
    ident = np.eye(P, dtype=np.float32).astype(ml_dtypes.bfloat16)
    w1 = np.asarray(W1, np.float32).astype(ml_dtypes.bfloat16)
    w2 = np.asarray(W2, np.float32).astype(ml_dtypes.bfloat16)
    b1c = np.asarray(b1, np.float32).reshape(-1)
    b2c = np.asarray(b2, np.float32).reshape(-1)
    hb1 = bool(np.any(b1c != 0.0))
    hb2 = bool(np.any(b2c != 0.0))

    nc1 = build_layer_program(cfg, meta["ovf_cols"], layer=1, has_bias=hb1)
    in_maps = [
        {"src_tab": xs, "w": w1, "iota": iota, "ident": ident,
         "self_rows": np.ascontiguousarray(xs[c * npc:(c + 1) * npc]),
         **{k: pc[k] for k in ("idx_tab", "d_tab", "dinv_cols")}}
        for c, pc in enumerate(meta["per_core"])
    ]
    if hb1:
        bf = np.ascontiguousarray(np.broadcast_to(b1c, (P, cfg.out1_f)).astype(np.float32))
        for m in in_maps:
            m["bias_full"] = bf
    res1 = runner(nc1, in_maps)

    gs = np.zeros((cfg.tab_rows, P), dtype=ml_dtypes.bfloat16)
    for c in range(cfg.n_cores):
        gs[c * npc:(c + 1) * npc] = res1[c]["out_t"]

    nc2 = build_layer_program(cfg, meta["ovf_cols"], layer=2, has_bias=hb2)
    for c in range(cfg.n_cores):
        in_maps[c] = dict(in_maps[c])
        in_maps[c]["src_tab"] = gs
        in_maps[c]["self_rows"] = np.ascontiguousarray(gs[c * npc:(c + 1) * npc])
        in_maps[c]["w"] = w2
        in_maps[c].pop("bias_full", None)
        if hb2:
            in_maps[c]["bias_full"] = np.ascontiguousarray(
                np.broadcast_to(b2c, (P, cfg.out2_f)).astype(np.float32))
    res2 = runner(nc2, in_maps)

    out = np.zeros((cfg.n_pad, cfg.out2_f), dtype=np.float32)
    for c in range(cfg.n_cores):
        out[c * npc:(c + 1) * npc] = res2[c]["out_t"]
    out = out[:cfg.n_nodes]
    if want_times and times is not None:
        return out, times
    return out


def kernel(x, edge_index, W1, b1, W2, b2):
    return _run_gcn(x, edge_index, W1, b1, W2, b2, FULL)
